# revision 12
# baseline (speedup 1.0000x reference)
"""GPS layer (GCN + dense Performer attention + FFN) on 8 Trainium2 cores.

Strategy (per core, rows R=1024 of N=8192 nodes):
  - GCN segment-sum is done as a dense matmul with the normalized adjacency
    A = D^-1/2 (Adj + I) D^-1/2, built host-side and shipped bf16 in an
    lhsT-friendly layout [rb, n_inner, n_chunk, r_inner].
  - All activations that feed matmul lhsT positions are produced transposed
    (feature-major) directly, so only small [128,128] PE transposes are needed.
  - Attention is computed in transposed score layout ST[c, r] = kf@qf^T so that
    softmax needs no max-subtraction (scores bounded ~15), the denominator
    comes from an appended ones-column of V, and exp(ST) tiles are directly the
    lhsT of the P@V matmul (no [N,N] transposes, nothing N^2 hits HBM).
  - kf^T and V are all-gathered across the 8 cores on-device, in TWO halves:
    half A fires while the GCN matmuls for rows 512..1023 still run, so the
    collective latency hides under PE work; attention then visits half-A
    column chunks first.
  - exp() is batched 2 score-chunks at a time ([128,1024] ACTIVATE) to
    amortize the ~352-cycle ACT fixed overhead; ReLU runs on DVE.
"""

import os
import sys

sys.path.insert(0, "/opt/trn_rl_repo")
os.environ.setdefault("MYCRO_LOCAL_CACHE", "1")

import numpy as np
import ml_dtypes

import concourse.bass as bass
import concourse.tile as tile
from concourse import bacc, mybir
from concourse.bass_utils import run_bass_kernel_spmd
from concourse.masks import make_identity

f32 = mybir.dt.float32
bf16 = mybir.dt.bfloat16
f8 = mybir.dt.float8e3
DR = mybir.MatmulPerfMode.DoubleRow
BF = ml_dtypes.bfloat16

N, D, F, M = 8192, 256, 512, 256
NCORES = 8
R = N // NCORES          # rows per core (1024)
RB = R // 128            # row blocks per core (8)
KC = D // 128            # feature chunks (2)
NCH = N // 128           # node chunks (64)
FC = F // 128            # ffn chunks (4)
VA = 260                 # v free dim: 256 features + ones col + pad
EPS = 1e-5
RH = R // 2              # rows per collective half (512)


def _ln_block(nc, pool, x_sb, out_sb, g_bc, be_bc, eps_t):
    """LayerNorm over free dim (256) of x_sb [128, 256] f32 -> out_sb."""
    stats = pool.tile([128, 6], f32, tag="ln_stats")
    nc.vector.bn_stats(stats[:], x_sb)
    mv = pool.tile([128, 2], f32, tag="ln_mv")
    nc.vector.bn_aggr(mv[:], stats[:])
    # rstd = 1/sqrt(var + eps)
    nc.scalar.activation(mv[:, 1:2], mv[:, 1:2],
                         mybir.ActivationFunctionType.Sqrt, bias=eps_t)
    nc.vector.reciprocal(mv[:, 1:2], mv[:, 1:2])
    nc.vector.tensor_scalar(out=out_sb, in0=x_sb,
                            scalar1=mv[:, 0:1], scalar2=mv[:, 1:2],
                            op0=mybir.AluOpType.subtract,
                            op1=mybir.AluOpType.mult)
    nc.vector.tensor_mul(out_sb, out_sb, g_bc)
    nc.vector.tensor_add(out_sb, out_sb, be_bc)


def _build():
    nc = bacc.Bacc("TRN2", target_bir_lowering=False, debug=False,
                   num_devices=NCORES)

    def inp(name, shape, dt):
        return nc.dram_tensor(name, shape, dt, kind="ExternalInput")

    at_h = inp("at", [RB, 128, NCH, 128], bf16)   # at[rb,p,k,f] = A[R0+rb*128+f, k*128+p]
    ht_h = inp("ht", [D, N], bf16)                # h^T, full graph
    hres_h = inp("hres", [R, D], f32)             # h rows + b_gcn
    wgcn_h = inp("wgcn", [D, D], bf16)
    wq_h = inp("wq", [D, D], bf16)
    wk_h = inp("wk", [D, D], bf16)
    wv_h = inp("wv", [D, D], bf16)
    wo_h = inp("wo", [D, D], bf16)
    rft_h = inp("rft", [D, M], bf16)              # RF^T
    w1_h = inp("w1", [D, F], bf16)
    w2_h = inp("w2", [F, D], bf16)
    bq_h = inp("bqc", [D, 1], f32)
    bk_h = inp("bkc", [D, 1], f32)
    bvr_h = inp("bvr", [1, D], bf16)
    bor_h = inp("bor", [1, D], bf16)
    b1r_h = inp("b1r", [1, F], bf16)
    b2r_h = inp("b2r", [1, D], bf16)
    gb_h = {}
    for nm in ("g1", "be1", "g2", "be2", "g3", "be3"):
        gb_h[nm] = inp(nm, [1, D], f32)

    out_h = nc.dram_tensor("out", [R, D], f32, kind="ExternalOutput")

    with tile.TileContext(nc) as tc:
        _body(tc, at_h, ht_h, hres_h, wgcn_h, wq_h, wk_h, wv_h, wo_h, rft_h,
              w1_h, w2_h, bq_h, bk_h, bvr_h, bor_h, b1r_h, b2r_h, gb_h, out_h)
    nc.compile()
    return nc


def _body(tc, at_h, ht_h, hres_h, wgcn_h, wq_h, wk_h, wv_h, wo_h, rft_h,
          w1_h, w2_h, bq_h, bk_h, bvr_h, bor_h, b1r_h, b2r_h, gb_h, out_h):
    from contextlib import ExitStack
    nc = tc.nc
    Exp = mybir.ActivationFunctionType.Exp

    with ExitStack() as octx:
        const = octx.enter_context(tc.tile_pool(name="const", bufs=1))
        persist = octx.enter_context(tc.tile_pool(name="persist", bufs=1))
        dram = octx.enter_context(tc.tile_pool(name="dram", bufs=1, space="DRAM"))

        # ---- constants ----
        def wtile(h, chunks, width, name):
            t = const.tile([128, chunks, width], bf16, tag=name)
            nc.sync.dma_start(
                t[:], h[:, :].rearrange("(c p) w -> p c w", p=128))
            return t

        wgcn_sb = wtile(wgcn_h, KC, D, "wgcn")
        wq_sb = wtile(wq_h, KC, D, "wq")
        wk_sb = wtile(wk_h, KC, D, "wk")
        wv_sb = wtile(wv_h, KC, D, "wv")
        wo_sb = wtile(wo_h, KC, D, "wo")
        rft_sb = wtile(rft_h, KC, M, "rft")
        w1_sb = wtile(w1_h, KC, F, "w1")
        w2_sb = wtile(w2_h, FC, D, "w2")

        bq_sb = const.tile([128, KC], f32, tag="bq")
        bk_sb = const.tile([128, KC], f32, tag="bk")
        for j in range(KC):
            nc.sync.dma_start(bq_sb[:, j:j + 1], bq_h[j * 128:(j + 1) * 128, :])
            nc.sync.dma_start(bk_sb[:, j:j + 1], bk_h[j * 128:(j + 1) * 128, :])
        bvr_sb = const.tile([1, D], bf16, tag="bvr")
        nc.sync.dma_start(bvr_sb[:], bvr_h[:, :])
        bor_sb = const.tile([1, D], bf16, tag="bor")
        nc.sync.dma_start(bor_sb[:], bor_h[:, :])
        b1r_sb = const.tile([1, F], bf16, tag="b1r")
        nc.sync.dma_start(b1r_sb[:], b1r_h[:, :])
        b2r_sb = const.tile([1, D], bf16, tag="b2r")
        nc.sync.dma_start(b2r_sb[:], b2r_h[:, :])

        gb_sb = {}
        for nm, h in gb_h.items():
            t = const.tile([128, D], f32, tag=nm)
            bcast = bass.AP(tensor=h.ap().tensor, offset=h.ap().offset,
                            ap=[[0, 128]] + list(h.ap().ap[1:]))
            nc.sync.dma_start(t[:], bcast)
            gb_sb[nm] = t

        ones_k1 = const.tile([1, 128], bf16, tag="ones")
        nc.vector.memset(ones_k1[:], 1.0)
        ident_bf = const.tile([128, 128], bf16, tag="ident")
        make_identity(nc, ident_bf[:])
        eps_t = const.tile([128, 1], f32, tag="eps")
        nc.vector.memset(eps_t[:], EPS)
        shift_t = const.tile([128, 1], f32, tag="shift")
        nc.vector.memset(shift_t[:], -12.0)

        # ---- persistent activations (kfa first: lives through both phases) ----
        kfa_sb = persist.tile([128, KC, N], bf16, tag="kfa")
        h1_sb = persist.tile([128, RB, D], f32, tag="h1")
        h1t_sb = persist.tile([128, KC, R], bf16, tag="h1t")
        qft_sb = persist.tile([128, KC, R], bf16, tag="qft")

        # ---- collective DRAM buffers (two halves) ----
        kft_loc = [dram.tile([M, RH], bf16, tag=f"kft_loc{h}", name=f"kft_loc{h}")
                   for h in range(2)]
        vaug_loc = [dram.tile([RH, VA], bf16, tag=f"vaug_loc{h}", name=f"vaug_loc{h}")
                    for h in range(2)]
        kft_all = [dram.tile([NCORES * M, RH], bf16, tag=f"kft_all{h}",
                              name=f"kft_all{h}", addr_space="Shared")
                   for h in range(2)]
        vaug_all = [dram.tile([NCORES * RH, VA], bf16, tag=f"vaug_all{h}",
                               name=f"vaug_all{h}", addr_space="Shared")
                    for h in range(2)]

        # ============ Phase 1: GCN + qkv, collectives fired per half ============
        with ExitStack() as p1:
            gcn = p1.enter_context(tc.tile_pool(name="gcn", bufs=1))
            ht_sb = gcn.tile([128, KC, N], bf16, tag="ht")
            xw_sb = gcn.tile([128, NCH, D], bf16, tag="xw")
            hres_sb = gcn.tile([128, RB, D], f32, tag="hres")
            nc.sync.dma_start(
                hres_sb[:], hres_h[:, :].rearrange("(rb p) d -> p rb d", p=128))
            qkvb = p1.enter_context(tc.tile_pool(name="qkvb", bufs=1))
            kt_sb = qkvb.tile([128, KC, R], bf16, tag="kt")
            qt_sb = qkvb.tile([128, KC, R], bf16, tag="qt")
            kft_sb = qkvb.tile([128, KC, R], bf16, tag="kft")
            atp = p1.enter_context(tc.tile_pool(name="atp", bufs=6))
            sc1 = p1.enter_context(tc.tile_pool(name="sc1", bufs=4))
            mm_ps = p1.enter_context(tc.tile_pool(name="mm_ps", bufs=3, space="PSUM"))
            hloc_ps = p1.enter_context(tc.tile_pool(name="hloc_ps", bufs=2, space="PSUM"))
            tp_ps = p1.enter_context(tc.tile_pool(name="tp_ps", bufs=2, space="PSUM"))

            # ht in [128,2048] slices so the first xw matmul starts early;
            # issued from the gpsimd queue to keep the sync queue free
            for j in range(KC):
                for sl in range(N // 2048):
                    nc.gpsimd.dma_start(ht_sb[:, j, sl * 2048:(sl + 1) * 2048],
                                        ht_h[j * 128:(j + 1) * 128,
                                             sl * 2048:(sl + 1) * 2048])

            # xw = h @ W_gcn for the whole graph (node-major), bf16
            for j in range(NCH):
                ps = mm_ps.tile([128, 512], f32, tag="mm")
                for jj in range(KC):
                    nc.tensor.matmul(ps[:, 0:D],
                                     ht_sb[:, jj, j * 128:(j + 1) * 128],
                                     wgcn_sb[:, jj, :],
                                     start=(jj == 0), stop=(jj == KC - 1))
                nc.vector.tensor_copy(xw_sb[:, j, :], ps[:, 0:D])

            def fire_kft(half):
                # kf^T all-gather: scores on the other cores' chunks are the
                # first consumers, so this fires as early as its input allows
                r0 = half * RH
                nc.gpsimd.collective_compute(
                    "AllGather", mybir.AluOpType.bypass,
                    replica_groups=[list(range(NCORES))],
                    ins=[kft_loc[half][:].opt()], outs=[kft_all[half][:].opt()])
                for mc in range(KC):
                    nc.sync.dma_start(
                        kfa_sb[:, mc, :].rearrange(
                            "p (c rr) -> p c rr", c=NCORES)[:, :, r0:r0 + RH],
                        kft_all[half][:, :].rearrange(
                            "(c m p) r -> m p c r", m=KC, p=128)[mc])

            def fire_vaug(half):
                nc.gpsimd.collective_compute(
                    "AllGather", mybir.AluOpType.bypass,
                    replica_groups=[list(range(NCORES))],
                    ins=[vaug_loc[half][:].opt()], outs=[vaug_all[half][:].opt()])

            def qkv_half(half, fire=True):
                r0 = half * RH
                # kT first (feature-major) for rows [r0, r0+RH)
                for (w_sb, b_sb, dst, scale) in ((wk_sb, bk_sb, kt_sb, None),):
                    for jj in range(KC):
                        ps = mm_ps.tile([128, 512], f32, tag="mm")
                        for j in range(KC):
                            nc.tensor.matmul(
                                ps[:],
                                w_sb[:, j, jj * 128:(jj + 1) * 128],
                                h1t_sb[:, j, r0:r0 + RH],
                                start=(j == 0), stop=(j == KC - 1))
                        if scale is None:
                            nc.vector.tensor_scalar(
                                out=dst[:, jj, r0:r0 + RH], in0=ps[:],
                                scalar1=b_sb[:, jj:jj + 1], scalar2=None,
                                op0=mybir.AluOpType.add)
                        else:
                            nc.vector.tensor_scalar(
                                out=dst[:, jj, r0:r0 + RH], in0=ps[:],
                                scalar1=b_sb[:, jj:jj + 1], scalar2=scale,
                                op0=mybir.AluOpType.add, op1=mybir.AluOpType.mult)
                def rf_proj(src, dst, store_kft):
                    for mc in range(KC):
                        ps = mm_ps.tile([128, 512], f32, tag="mm")
                        for j in range(KC):
                            nc.tensor.matmul(
                                ps[:],
                                rft_sb[:, j, mc * 128:(mc + 1) * 128],
                                src[:, j, r0:r0 + RH],
                                start=(j == 0), stop=(j == KC - 1))
                        nc.vector.tensor_copy(dst[:, mc, r0:r0 + RH], ps[:])
                        if store_kft:
                            nc.sync.dma_start(
                                kft_loc[half][mc * 128:(mc + 1) * 128, :],
                                dst[:, mc, r0:r0 + RH])

                rf_proj(kt_sb, kft_sb, True)
                if fire:
                    fire_kft(half)
                # now qT and the random-feature projection of q
                for (w_sb, b_sb, dst, scale) in ((wq_sb, bq_sb, qt_sb, None),):
                    for jj in range(KC):
                        ps = mm_ps.tile([128, 512], f32, tag="mm")
                        for j in range(KC):
                            nc.tensor.matmul(
                                ps[:],
                                w_sb[:, j, jj * 128:(jj + 1) * 128],
                                h1t_sb[:, j, r0:r0 + RH],
                                start=(j == 0), stop=(j == KC - 1))
                        nc.vector.tensor_scalar(
                            out=dst[:, jj, r0:r0 + RH], in0=ps[:],
                            scalar1=b_sb[:, jj:jj + 1], scalar2=None,
                            op0=mybir.AluOpType.add)
                rf_proj(qt_sb, qft_sb, False)
                # v rows (node-major) + ones column
                for rb in range(half * (RB // 2), (half + 1) * (RB // 2)):
                    ps = mm_ps.tile([128, 512], f32, tag="mm")
                    for j in range(KC):
                        nc.tensor.matmul(ps[:, 0:D],
                                         h1t_sb[:, j, rb * 128:(rb + 1) * 128],
                                         wv_sb[:, j, :], start=(j == 0), stop=False)
                    nc.tensor.matmul(ps[:, 0:D], ones_k1[:], bvr_sb[:],
                                     start=False, stop=True)
                    vt = sc1.tile([128, VA], bf16, tag="vaug")
                    nc.vector.tensor_copy(vt[:, 0:D], ps[:, 0:D])
                    nc.vector.memset(vt[:, D:D + 1], 1.0)
                    nc.vector.memset(vt[:, D + 1:VA], 0.0)
                    nc.sync.dma_start(
                        vaug_loc[half][(rb - half * (RB // 2)) * 128:
                                       (rb - half * (RB // 2) + 1) * 128, :],
                        vt[:])
                if fire:
                    fire_vaug(half)

            # GCN row blocks; after each half of rows, produce q/k/v + gather
            for rb in range(RB):
                ps = hloc_ps.tile([128, D], f32, tag="hloc")
                for jj in range(NCH // 8):
                    at_t = atp.tile([128, 8, 128], bf16, tag="at")
                    nc.scalar.dma_start(at_t[:], at_h[rb, :, jj * 8:(jj + 1) * 8, :])
                    for k in range(8):
                        j = jj * 8 + k
                        nc.tensor.matmul(ps[:], at_t[:, k, :], xw_sb[:, j, :],
                                         start=(j == 0), stop=(j == NCH - 1))
                x1 = sc1.tile([128, D], f32, tag="x1")
                nc.vector.tensor_add(x1[:], ps[:], hres_sb[:, rb, :])
                _ln_block(nc, sc1, x1[:], h1_sb[:, rb, :],
                          gb_sb["g1"][:], gb_sb["be1"][:], eps_t[:])
                h1bf = sc1.tile([128, D], bf16, tag="h1bf")
                nc.vector.tensor_copy(h1bf[:], h1_sb[:, rb, :])
                for j in range(KC):
                    tp = tp_ps.tile([128, 128], bf16, tag="tp1")
                    nc.tensor.transpose(tp[:], h1bf[:, j * 128:(j + 1) * 128],
                                        ident_bf[:])
                    nc.vector.tensor_copy(
                        h1t_sb[:, j, rb * 128:(rb + 1) * 128], tp[:])
                if rb == RB // 2 - 1:
                    qkv_half(0)
                if rb == RB - 1:
                    qkv_half(1, fire=False)

        # half-B collectives fire outside the phase-1 pool scope: pool-release
        # instructions on the gpsimd queue must not sit behind a blocked
        # collective trigger (head-of-line stall seen in the trace)
        fire_kft(1)
        fire_vaug(1)

        # ============ Phase 2: attention + FFN ============
        with ExitStack() as p3:
            glob = p3.enter_context(tc.tile_pool(name="glob", bufs=1))
            slabs = p3.enter_context(tc.tile_pool(name="slabs", bufs=1))
            sc3 = p3.enter_context(tc.tile_pool(name="sc3", bufs=3))
            st_ps = p3.enter_context(tc.tile_pool(name="st_ps", bufs=2, space="PSUM"))
            num_ps = p3.enter_context(tc.tile_pool(name="num_ps", bufs=1, space="PSUM"))
            tp2_ps = p3.enter_context(tc.tile_pool(name="tp2_ps", bufs=1, space="PSUM"))
            sm_ps = p3.enter_context(tc.tile_pool(name="sm_ps", bufs=2, space="PSUM"))

            vaug_sb = glob.tile([128, NCH, VA], bf16, tag="vaa")
            # cg visit order: all half-A chunks (local rows 0..511 of every
            # core) first, then half-B — matches collective arrival order.
            cg_order = []
            for half in range(2):
                for c in range(NCORES):
                    for l in range(4):
                        cg_order.append((c * 8 + half * 4 + l, half, c, l))
                # gathered V loads for this half: one strided DMA per core
                for c in range(NCORES):
                    cg0 = c * 8 + half * 4
                    nc.sync.dma_start(
                        vaug_sb[:, cg0:cg0 + 4, :],
                        vaug_all[half][c * 4 * 128:(c * 4 + 4) * 128, :].rearrange(
                            "(l p) v -> p l v", p=128))

            RC = 512  # r-chunk for score slabs
            for rc in range(R // RC):
                slab = slabs.tile([128, NCH, RC], bf16, tag="slab")
                for ci in range(0, NCH, 2):
                    ps = st_ps.tile([128, 2, RC], f32, tag="st")
                    for t in range(2):
                        cg = cg_order[ci + t][0]
                        for j in range(KC):
                            nc.tensor.matmul(
                                ps[:, t, :],
                                kfa_sb[:, j, cg * 128:(cg + 1) * 128],
                                qft_sb[:, j, rc * RC:(rc + 1) * RC],
                                start=(j == 0), stop=(j == KC - 1))
                    # batched exp over 2 chunks: [128, 1024] ACTIVATE.
                    # exp(raw/16 - 12): the /16 is the 1/sqrt(D) score scale;
                    # the -12 shift keeps exp within fp8-e4m3 range (max score
                    # ~15.1 -> exp <= e^3.1 ~ 22 << 448) and cancels in the
                    # softmax ratio exactly.
                    cg0 = cg_order[ci][0]
                    nc.scalar.activation(slab[:, cg0:cg0 + 2, :], ps[:], Exp,
                                         scale=1.0 / 16.0)

                for half in range(RC // 128):
                    rb = rc * (RC // 128) + half
                    nps = num_ps.tile([128, VA], f32, tag="num")
                    for i, (cg, _, _, _) in enumerate(cg_order):
                        nc.tensor.matmul(
                            nps[:], slab[:, cg, half * 128:(half + 1) * 128],
                            vaug_sb[:, cg, :],
                            start=(i == 0), stop=(i == NCH - 1))
                    rec = sc3.tile([128, 1], f32, tag="rec")
                    nc.vector.reciprocal(rec[:], nps[:, D:D + 1])
                    attn_bf = sc3.tile([128, D], bf16, tag="attn")
                    nc.vector.tensor_scalar_mul(attn_bf[:], nps[:, 0:D], rec[:])
                    attnT = sc3.tile([128, KC, 128], bf16, tag="attnT")
                    for j in range(KC):
                        tp = tp2_ps.tile([128, 128], bf16, tag="tp2")
                        nc.tensor.transpose(
                            tp[:], attn_bf[:, j * 128:(j + 1) * 128], ident_bf[:])
                        nc.vector.tensor_copy(attnT[:, j, :], tp[:])

                    hg = sm_ps.tile([128, F], f32, tag="smz")
                    for j in range(KC):
                        nc.tensor.matmul(hg[:, 0:D], attnT[:, j, :], wo_sb[:, j, :],
                                         start=(j == 0), stop=False)
                    nc.tensor.matmul(hg[:, 0:D], ones_k1[:], bor_sb[:],
                                     start=False, stop=True)
                    x2 = sc3.tile([128, D], f32, tag="x2")
                    nc.vector.tensor_add(x2[:], hg[:, 0:D], h1_sb[:, rb, :])
                    h2 = sc3.tile([128, D], f32, tag="h2")
                    _ln_block(nc, sc3, x2[:], h2[:],
                              gb_sb["g2"][:], gb_sb["be2"][:], eps_t[:])
                    h2bf = sc3.tile([128, D], bf16, tag="h2bf")
                    nc.vector.tensor_copy(h2bf[:], h2[:])
                    h2T = sc3.tile([128, KC, 128], bf16, tag="h2T")
                    for j in range(KC):
                        tp = tp2_ps.tile([128, 128], bf16, tag="tp2")
                        nc.tensor.transpose(
                            tp[:], h2bf[:, j * 128:(j + 1) * 128], ident_bf[:])
                        nc.vector.tensor_copy(h2T[:, j, :], tp[:])

                    ups = sm_ps.tile([128, F], f32, tag="smz")
                    for j in range(KC):
                        nc.tensor.matmul(ups[:], h2T[:, j, :], w1_sb[:, j, :],
                                         start=(j == 0), stop=False)
                    nc.tensor.matmul(ups[:], ones_k1[:], b1r_sb[:],
                                     start=False, stop=True)
                    u_sb = sc3.tile([128, F], bf16, tag="usb")
                    # ReLU on DVE (keeps ACT tables on exp/sqrt only)
                    nc.vector.tensor_scalar_max(u_sb[:], ups[:], 0.0)
                    uT = sc3.tile([128, FC, 128], bf16, tag="uT")
                    for jf in range(FC):
                        tp = tp2_ps.tile([128, 128], bf16, tag="tp2")
                        nc.tensor.transpose(
                            tp[:], u_sb[:, jf * 128:(jf + 1) * 128], ident_bf[:])
                        nc.vector.tensor_copy(uT[:, jf, :], tp[:])

                    o2 = sm_ps.tile([128, F], f32, tag="smz")
                    for jf in range(FC):
                        nc.tensor.matmul(o2[:, 0:D], uT[:, jf, :], w2_sb[:, jf, :],
                                         start=(jf == 0), stop=False)
                    nc.tensor.matmul(o2[:, 0:D], ones_k1[:], b2r_sb[:],
                                     start=False, stop=True)
                    x3 = sc3.tile([128, D], f32, tag="x3")
                    nc.vector.tensor_add(x3[:], o2[:, 0:D], h2[:])
                    o_sb = sc3.tile([128, D], f32, tag="osb")
                    _ln_block(nc, sc3, x3[:], o_sb[:],
                              gb_sb["g3"][:], gb_sb["be3"][:], eps_t[:])
                    nc.sync.dma_start(out_h[rb * 128:(rb + 1) * 128, :], o_sb[:])


_NC_CACHE = None


def _get_nc():
    global _NC_CACHE
    if _NC_CACHE is None:
        _NC_CACHE = _build()
    return _NC_CACHE


def _host_prep(inputs):
    """Build per-core in_maps from full inputs."""
    h = np.ascontiguousarray(np.asarray(inputs["h"], dtype=np.float32))
    ei = np.asarray(inputs["edge_index"]).astype(np.int64)
    src, dst = ei[0], ei[1]

    deg = np.bincount(dst, minlength=N).astype(np.float32) + 1.0
    dinv = 1.0 / np.sqrt(deg)
    coef = (dinv[src] * dinv[dst]).astype(np.float32)
    A = np.zeros((N, N), np.float32)
    np.add.at(A, (dst, src), coef)
    idx = np.arange(N)
    A[idx, idx] += dinv * dinv

    f32c = lambda k: np.ascontiguousarray(np.asarray(inputs[k], dtype=np.float32))
    bfc = lambda x: np.ascontiguousarray(x.astype(BF))

    ht = bfc(h.T)
    w = {k: f32c(k) for k in ("W_gcn", "Wq", "Wk", "Wv", "Wo", "RF",
                              "W1", "W2", "b_gcn", "bq", "bk", "bv", "bo",
                              "b1", "b2", "g1", "be1", "g2", "be2", "g3", "be3")}

    common = {
        "ht": ht,
        "wgcn": bfc(w["W_gcn"]), "wq": bfc(w["Wq"]), "wk": bfc(w["Wk"]),
        "wv": bfc(w["Wv"]), "wo": bfc(w["Wo"]), "rft": bfc(w["RF"].T),
        "w1": bfc(w["W1"]), "w2": bfc(w["W2"]),
        "bqc": np.ascontiguousarray(w["bq"].reshape(D, 1)),
        "bkc": np.ascontiguousarray(w["bk"].reshape(D, 1)),
        "bvr": bfc(w["bv"].reshape(1, D)),
        "bor": bfc(w["bo"].reshape(1, D)),
        "b1r": bfc(w["b1"].reshape(1, F)),
        "b2r": bfc(w["b2"].reshape(1, D)),
        "g1": np.ascontiguousarray(w["g1"].reshape(1, D)),
        "be1": np.ascontiguousarray(w["be1"].reshape(1, D)),
        "g2": np.ascontiguousarray(w["g2"].reshape(1, D)),
        "be2": np.ascontiguousarray(w["be2"].reshape(1, D)),
        "g3": np.ascontiguousarray(w["g3"].reshape(1, D)),
        "be3": np.ascontiguousarray(w["be3"].reshape(1, D)),
    }

    in_maps = []
    for c in range(NCORES):
        r0 = c * R
        # at[rb, p, k, f] = A[r0 + rb*128 + f, k*128 + p]
        a_loc = A[r0:r0 + R].reshape(RB, 128, NCH, 128)
        at = np.ascontiguousarray(a_loc.transpose(0, 3, 2, 1).astype(BF))
        m = dict(common)
        m["at"] = at
        m["hres"] = np.ascontiguousarray(h[r0:r0 + R] + w["b_gcn"])
        in_maps.append(m)
    return in_maps


def kernel(**inputs):
    nc = _get_nc()
    in_maps = _host_prep(inputs)
    res = run_bass_kernel_spmd(nc, in_maps, core_ids=list(range(NCORES)))
    out = np.concatenate([np.asarray(r["out"]) for r in res.results], axis=0)
    return out.astype(np.float32)


# revision 13
# speedup vs baseline: 1.0006x; 1.0006x over previous
"""GPS layer (GCN + dense Performer attention + FFN) on 8 Trainium2 cores.

Strategy (per core, rows R=1024 of N=8192 nodes):
  - GCN segment-sum is done as a dense matmul with the normalized adjacency
    A = D^-1/2 (Adj + I) D^-1/2, built host-side and shipped bf16 in an
    lhsT-friendly layout [rb, n_inner, n_chunk, r_inner].
  - All activations that feed matmul lhsT positions are produced transposed
    (feature-major) directly, so only small [128,128] PE transposes are needed.
  - Attention is computed in transposed score layout ST[c, r] = kf@qf^T so that
    softmax needs no max-subtraction (scores bounded ~15), the denominator
    comes from an appended ones-column of V, and exp(ST) tiles are directly the
    lhsT of the P@V matmul (no [N,N] transposes, nothing N^2 hits HBM).
  - kf^T and V are all-gathered across the 8 cores on-device, in TWO halves:
    half A fires while the GCN matmuls for rows 512..1023 still run, so the
    collective latency hides under PE work; attention then visits half-A
    column chunks first.
  - exp() is batched 2 score-chunks at a time ([128,1024] ACTIVATE) to
    amortize the ~352-cycle ACT fixed overhead; ReLU runs on DVE.
"""

import os
import sys

sys.path.insert(0, "/opt/trn_rl_repo")
os.environ.setdefault("MYCRO_LOCAL_CACHE", "1")

import numpy as np
import ml_dtypes

import concourse.bass as bass
import concourse.tile as tile
from concourse import bacc, mybir
from concourse.bass_utils import run_bass_kernel_spmd
from concourse.masks import make_identity

f32 = mybir.dt.float32
bf16 = mybir.dt.bfloat16
f8 = mybir.dt.float8e3
DR = mybir.MatmulPerfMode.DoubleRow
BF = ml_dtypes.bfloat16

N, D, F, M = 8192, 256, 512, 256
NCORES = 8
R = N // NCORES          # rows per core (1024)
RB = R // 128            # row blocks per core (8)
KC = D // 128            # feature chunks (2)
NCH = N // 128           # node chunks (64)
FC = F // 128            # ffn chunks (4)
VA = 260                 # v free dim: 256 features + ones col + pad
EPS = 1e-5
RH = R // 2              # rows per collective half (512)


def _ln_block(nc, pool, x_sb, out_sb, g_bc, be_bc, eps_t):
    """LayerNorm over free dim (256) of x_sb [128, 256] f32 -> out_sb."""
    stats = pool.tile([128, 6], f32, tag="ln_stats")
    nc.vector.bn_stats(stats[:], x_sb)
    mv = pool.tile([128, 2], f32, tag="ln_mv")
    nc.vector.bn_aggr(mv[:], stats[:])
    # rstd = 1/sqrt(var + eps)
    nc.scalar.activation(mv[:, 1:2], mv[:, 1:2],
                         mybir.ActivationFunctionType.Sqrt, bias=eps_t)
    nc.vector.reciprocal(mv[:, 1:2], mv[:, 1:2])
    nc.vector.tensor_scalar(out=out_sb, in0=x_sb,
                            scalar1=mv[:, 0:1], scalar2=mv[:, 1:2],
                            op0=mybir.AluOpType.subtract,
                            op1=mybir.AluOpType.mult)
    nc.vector.tensor_mul(out_sb, out_sb, g_bc)
    nc.vector.tensor_add(out_sb, out_sb, be_bc)


def _build():
    nc = bacc.Bacc("TRN2", target_bir_lowering=False, debug=False,
                   num_devices=NCORES)

    def inp(name, shape, dt):
        return nc.dram_tensor(name, shape, dt, kind="ExternalInput")

    at_h = inp("at", [RB, 128, NCH, 128], bf16)   # at[rb,p,k,f] = A[R0+rb*128+f, k*128+p]
    ht_h = inp("ht", [D, N], bf16)                # h^T, full graph
    hres_h = inp("hres", [R, D], f32)             # h rows + b_gcn
    wgcn_h = inp("wgcn", [D, D], bf16)
    wq_h = inp("wq", [D, D], bf16)
    wk_h = inp("wk", [D, D], bf16)
    wv_h = inp("wv", [D, D], bf16)
    wo_h = inp("wo", [D, D], bf16)
    rft_h = inp("rft", [D, M], bf16)              # RF^T
    w1_h = inp("w1", [D, F], bf16)
    w2_h = inp("w2", [F, D], bf16)
    bq_h = inp("bqc", [D, 1], f32)
    bk_h = inp("bkc", [D, 1], f32)
    bvr_h = inp("bvr", [1, D], bf16)
    bor_h = inp("bor", [1, D], bf16)
    b1r_h = inp("b1r", [1, F], bf16)
    b2r_h = inp("b2r", [1, D], bf16)
    gb_h = {}
    for nm in ("g1", "be1", "g2", "be2", "g3", "be3"):
        gb_h[nm] = inp(nm, [1, D], f32)

    out_h = nc.dram_tensor("out", [R, D], f32, kind="ExternalOutput")

    with tile.TileContext(nc) as tc:
        _body(tc, at_h, ht_h, hres_h, wgcn_h, wq_h, wk_h, wv_h, wo_h, rft_h,
              w1_h, w2_h, bq_h, bk_h, bvr_h, bor_h, b1r_h, b2r_h, gb_h, out_h)
    nc.compile()
    return nc


def _body(tc, at_h, ht_h, hres_h, wgcn_h, wq_h, wk_h, wv_h, wo_h, rft_h,
          w1_h, w2_h, bq_h, bk_h, bvr_h, bor_h, b1r_h, b2r_h, gb_h, out_h):
    from contextlib import ExitStack
    nc = tc.nc
    Exp = mybir.ActivationFunctionType.Exp

    with ExitStack() as octx:
        const = octx.enter_context(tc.tile_pool(name="const", bufs=1))
        persist = octx.enter_context(tc.tile_pool(name="persist", bufs=1))
        dram = octx.enter_context(tc.tile_pool(name="dram", bufs=1, space="DRAM"))

        # ---- constants ----
        def wtile(h, chunks, width, name):
            t = const.tile([128, chunks, width], bf16, tag=name)
            nc.sync.dma_start(
                t[:], h[:, :].rearrange("(c p) w -> p c w", p=128))
            return t

        wgcn_sb = wtile(wgcn_h, KC, D, "wgcn")
        wq_sb = wtile(wq_h, KC, D, "wq")
        wk_sb = wtile(wk_h, KC, D, "wk")
        wv_sb = wtile(wv_h, KC, D, "wv")
        wo_sb = wtile(wo_h, KC, D, "wo")
        rft_sb = wtile(rft_h, KC, M, "rft")
        w1_sb = wtile(w1_h, KC, F, "w1")
        w2_sb = wtile(w2_h, FC, D, "w2")

        bq_sb = const.tile([128, KC], f32, tag="bq")
        bk_sb = const.tile([128, KC], f32, tag="bk")
        for j in range(KC):
            nc.sync.dma_start(bq_sb[:, j:j + 1], bq_h[j * 128:(j + 1) * 128, :])
            nc.sync.dma_start(bk_sb[:, j:j + 1], bk_h[j * 128:(j + 1) * 128, :])
        bvr_sb = const.tile([1, D], bf16, tag="bvr")
        nc.sync.dma_start(bvr_sb[:], bvr_h[:, :])
        bor_sb = const.tile([1, D], bf16, tag="bor")
        nc.sync.dma_start(bor_sb[:], bor_h[:, :])
        b1r_sb = const.tile([1, F], bf16, tag="b1r")
        nc.sync.dma_start(b1r_sb[:], b1r_h[:, :])
        b2r_sb = const.tile([1, D], bf16, tag="b2r")
        nc.sync.dma_start(b2r_sb[:], b2r_h[:, :])

        gb_sb = {}
        for nm, h in gb_h.items():
            t = const.tile([128, D], f32, tag=nm)
            bcast = bass.AP(tensor=h.ap().tensor, offset=h.ap().offset,
                            ap=[[0, 128]] + list(h.ap().ap[1:]))
            nc.sync.dma_start(t[:], bcast)
            gb_sb[nm] = t

        ones_k1 = const.tile([1, 128], bf16, tag="ones")
        nc.vector.memset(ones_k1[:], 1.0)
        ident_bf = const.tile([128, 128], bf16, tag="ident")
        make_identity(nc, ident_bf[:])
        eps_t = const.tile([128, 1], f32, tag="eps")
        nc.vector.memset(eps_t[:], EPS)
        shift_t = const.tile([128, 1], f32, tag="shift")
        nc.vector.memset(shift_t[:], -12.0)

        # ---- persistent activations (kfa first: lives through both phases) ----
        kfa_sb = persist.tile([128, KC, N], bf16, tag="kfa")
        h1_sb = persist.tile([128, RB, D], f32, tag="h1")
        h1t_sb = persist.tile([128, KC, R], bf16, tag="h1t")
        qft_sb = persist.tile([128, KC, R], bf16, tag="qft")

        # ---- collective DRAM buffers (two halves) ----
        kft_loc = [dram.tile([M, RH], bf16, tag=f"kft_loc{h}", name=f"kft_loc{h}")
                   for h in range(2)]
        vaug_loc = [dram.tile([RH, VA], bf16, tag=f"vaug_loc{h}", name=f"vaug_loc{h}")
                    for h in range(2)]
        kft_all = [dram.tile([NCORES * M, RH], bf16, tag=f"kft_all{h}",
                              name=f"kft_all{h}", addr_space="Shared")
                   for h in range(2)]
        vaug_all = [dram.tile([NCORES * RH, VA], bf16, tag=f"vaug_all{h}",
                               name=f"vaug_all{h}", addr_space="Shared")
                    for h in range(2)]

        # ============ Phase 1: GCN + qkv, collectives fired per half ============
        with ExitStack() as p1:
            gcn = p1.enter_context(tc.tile_pool(name="gcn", bufs=1))
            ht_sb = gcn.tile([128, KC, N], bf16, tag="ht")
            xw_sb = gcn.tile([128, NCH, D], bf16, tag="xw")
            hres_sb = gcn.tile([128, RB, D], f32, tag="hres")
            nc.sync.dma_start(
                hres_sb[:], hres_h[:, :].rearrange("(rb p) d -> p rb d", p=128))
            qkvb = p1.enter_context(tc.tile_pool(name="qkvb", bufs=1))
            kt_sb = qkvb.tile([128, KC, R], bf16, tag="kt")
            qt_sb = qkvb.tile([128, KC, R], bf16, tag="qt")
            kft_sb = qkvb.tile([128, KC, R], bf16, tag="kft")
            atp = p1.enter_context(tc.tile_pool(name="atp", bufs=6))
            sc1 = p1.enter_context(tc.tile_pool(name="sc1", bufs=4))
            mm_ps = p1.enter_context(tc.tile_pool(name="mm_ps", bufs=3, space="PSUM"))
            hloc_ps = p1.enter_context(tc.tile_pool(name="hloc_ps", bufs=2, space="PSUM"))
            tp_ps = p1.enter_context(tc.tile_pool(name="tp_ps", bufs=2, space="PSUM"))

            # ht in [128,2048] slices so the first xw matmul starts early;
            # issued from the gpsimd queue to keep the sync queue free
            for j in range(KC):
                for sl in range(N // 2048):
                    nc.gpsimd.dma_start(ht_sb[:, j, sl * 2048:(sl + 1) * 2048],
                                        ht_h[j * 128:(j + 1) * 128,
                                             sl * 2048:(sl + 1) * 2048])

            # xw = h @ W_gcn for the whole graph (node-major), bf16
            for j in range(NCH):
                ps = mm_ps.tile([128, 512], f32, tag="mm")
                for jj in range(KC):
                    nc.tensor.matmul(ps[:, 0:D],
                                     ht_sb[:, jj, j * 128:(j + 1) * 128],
                                     wgcn_sb[:, jj, :],
                                     start=(jj == 0), stop=(jj == KC - 1))
                nc.vector.tensor_copy(xw_sb[:, j, :], ps[:, 0:D])

            def fire_kft(half):
                # kf^T all-gather: scores on the other cores' chunks are the
                # first consumers, so this fires as early as its input allows
                r0 = half * RH
                nc.gpsimd.collective_compute(
                    "AllGather", mybir.AluOpType.bypass,
                    replica_groups=[list(range(NCORES))],
                    ins=[kft_loc[half][:].opt()], outs=[kft_all[half][:].opt()])
                for mc in range(KC):
                    nc.gpsimd.dma_start(
                        kfa_sb[:, mc, :].rearrange(
                            "p (c rr) -> p c rr", c=NCORES)[:, :, r0:r0 + RH],
                        kft_all[half][:, :].rearrange(
                            "(c m p) r -> m p c r", m=KC, p=128)[mc])

            def fire_vaug(half):
                nc.gpsimd.collective_compute(
                    "AllGather", mybir.AluOpType.bypass,
                    replica_groups=[list(range(NCORES))],
                    ins=[vaug_loc[half][:].opt()], outs=[vaug_all[half][:].opt()])

            def qkv_half(half, fire=True):
                r0 = half * RH
                # kT first (feature-major) for rows [r0, r0+RH)
                for (w_sb, b_sb, dst, scale) in ((wk_sb, bk_sb, kt_sb, None),):
                    for jj in range(KC):
                        ps = mm_ps.tile([128, 512], f32, tag="mm")
                        for j in range(KC):
                            nc.tensor.matmul(
                                ps[:],
                                w_sb[:, j, jj * 128:(jj + 1) * 128],
                                h1t_sb[:, j, r0:r0 + RH],
                                start=(j == 0), stop=(j == KC - 1))
                        if scale is None:
                            nc.vector.tensor_scalar(
                                out=dst[:, jj, r0:r0 + RH], in0=ps[:],
                                scalar1=b_sb[:, jj:jj + 1], scalar2=None,
                                op0=mybir.AluOpType.add)
                        else:
                            nc.vector.tensor_scalar(
                                out=dst[:, jj, r0:r0 + RH], in0=ps[:],
                                scalar1=b_sb[:, jj:jj + 1], scalar2=scale,
                                op0=mybir.AluOpType.add, op1=mybir.AluOpType.mult)
                def rf_proj(src, dst, store_kft):
                    for mc in range(KC):
                        ps = mm_ps.tile([128, 512], f32, tag="mm")
                        for j in range(KC):
                            nc.tensor.matmul(
                                ps[:],
                                rft_sb[:, j, mc * 128:(mc + 1) * 128],
                                src[:, j, r0:r0 + RH],
                                start=(j == 0), stop=(j == KC - 1))
                        nc.vector.tensor_copy(dst[:, mc, r0:r0 + RH], ps[:])
                        if store_kft:
                            nc.sync.dma_start(
                                kft_loc[half][mc * 128:(mc + 1) * 128, :],
                                dst[:, mc, r0:r0 + RH])

                rf_proj(kt_sb, kft_sb, True)
                if fire:
                    fire_kft(half)
                # now qT and the random-feature projection of q
                for (w_sb, b_sb, dst, scale) in ((wq_sb, bq_sb, qt_sb, None),):
                    for jj in range(KC):
                        ps = mm_ps.tile([128, 512], f32, tag="mm")
                        for j in range(KC):
                            nc.tensor.matmul(
                                ps[:],
                                w_sb[:, j, jj * 128:(jj + 1) * 128],
                                h1t_sb[:, j, r0:r0 + RH],
                                start=(j == 0), stop=(j == KC - 1))
                        nc.vector.tensor_scalar(
                            out=dst[:, jj, r0:r0 + RH], in0=ps[:],
                            scalar1=b_sb[:, jj:jj + 1], scalar2=None,
                            op0=mybir.AluOpType.add)
                rf_proj(qt_sb, qft_sb, False)
                # v rows (node-major) + ones column
                for rb in range(half * (RB // 2), (half + 1) * (RB // 2)):
                    ps = mm_ps.tile([128, 512], f32, tag="mm")
                    for j in range(KC):
                        nc.tensor.matmul(ps[:, 0:D],
                                         h1t_sb[:, j, rb * 128:(rb + 1) * 128],
                                         wv_sb[:, j, :], start=(j == 0), stop=False)
                    nc.tensor.matmul(ps[:, 0:D], ones_k1[:], bvr_sb[:],
                                     start=False, stop=True)
                    vt = sc1.tile([128, VA], bf16, tag="vaug")
                    nc.vector.tensor_copy(vt[:, 0:D], ps[:, 0:D])
                    nc.vector.memset(vt[:, D:D + 1], 1.0)
                    nc.vector.memset(vt[:, D + 1:VA], 0.0)
                    nc.sync.dma_start(
                        vaug_loc[half][(rb - half * (RB // 2)) * 128:
                                       (rb - half * (RB // 2) + 1) * 128, :],
                        vt[:])
                if fire:
                    fire_vaug(half)

            # GCN row blocks; after each half of rows, produce q/k/v + gather
            for rb in range(RB):
                ps = hloc_ps.tile([128, D], f32, tag="hloc")
                for jj in range(NCH // 8):
                    at_t = atp.tile([128, 8, 128], bf16, tag="at")
                    nc.gpsimd.dma_start(at_t[:], at_h[rb, :, jj * 8:(jj + 1) * 8, :])
                    for k in range(8):
                        j = jj * 8 + k
                        nc.tensor.matmul(ps[:], at_t[:, k, :], xw_sb[:, j, :],
                                         start=(j == 0), stop=(j == NCH - 1))
                x1 = sc1.tile([128, D], f32, tag="x1")
                nc.vector.tensor_add(x1[:], ps[:], hres_sb[:, rb, :])
                _ln_block(nc, sc1, x1[:], h1_sb[:, rb, :],
                          gb_sb["g1"][:], gb_sb["be1"][:], eps_t[:])
                h1bf = sc1.tile([128, D], bf16, tag="h1bf")
                nc.vector.tensor_copy(h1bf[:], h1_sb[:, rb, :])
                for j in range(KC):
                    tp = tp_ps.tile([128, 128], bf16, tag="tp1")
                    nc.tensor.transpose(tp[:], h1bf[:, j * 128:(j + 1) * 128],
                                        ident_bf[:])
                    nc.vector.tensor_copy(
                        h1t_sb[:, j, rb * 128:(rb + 1) * 128], tp[:])
                if rb == RB // 2 - 1:
                    qkv_half(0)
                if rb == RB - 1:
                    qkv_half(1)

        # ============ Phase 2: attention + FFN ============
        with ExitStack() as p3:
            glob = p3.enter_context(tc.tile_pool(name="glob", bufs=1))
            slabs = p3.enter_context(tc.tile_pool(name="slabs", bufs=1))
            sc3 = p3.enter_context(tc.tile_pool(name="sc3", bufs=3))
            st_ps = p3.enter_context(tc.tile_pool(name="st_ps", bufs=2, space="PSUM"))
            num_ps = p3.enter_context(tc.tile_pool(name="num_ps", bufs=1, space="PSUM"))
            tp2_ps = p3.enter_context(tc.tile_pool(name="tp2_ps", bufs=1, space="PSUM"))
            sm_ps = p3.enter_context(tc.tile_pool(name="sm_ps", bufs=2, space="PSUM"))

            vaug_sb = glob.tile([128, NCH, VA], bf16, tag="vaa")
            # cg visit order: all half-A chunks (local rows 0..511 of every
            # core) first, then half-B — matches collective arrival order.
            cg_order = []
            for half in range(2):
                for c in range(NCORES):
                    for l in range(4):
                        cg_order.append((c * 8 + half * 4 + l, half, c, l))
                # gathered V loads for this half: one strided DMA per core
                for c in range(NCORES):
                    cg0 = c * 8 + half * 4
                    nc.gpsimd.dma_start(
                        vaug_sb[:, cg0:cg0 + 4, :],
                        vaug_all[half][c * 4 * 128:(c * 4 + 4) * 128, :].rearrange(
                            "(l p) v -> p l v", p=128))

            RC = 512  # r-chunk for score slabs
            for rc in range(R // RC):
                slab = slabs.tile([128, NCH, RC], bf16, tag="slab")
                for ci in range(0, NCH, 2):
                    ps = st_ps.tile([128, 2, RC], f32, tag="st")
                    for t in range(2):
                        cg = cg_order[ci + t][0]
                        for j in range(KC):
                            nc.tensor.matmul(
                                ps[:, t, :],
                                kfa_sb[:, j, cg * 128:(cg + 1) * 128],
                                qft_sb[:, j, rc * RC:(rc + 1) * RC],
                                start=(j == 0), stop=(j == KC - 1))
                    # batched exp over 2 chunks: [128, 1024] ACTIVATE.
                    # exp(raw/16 - 12): the /16 is the 1/sqrt(D) score scale;
                    # the -12 shift keeps exp within fp8-e4m3 range (max score
                    # ~15.1 -> exp <= e^3.1 ~ 22 << 448) and cancels in the
                    # softmax ratio exactly.
                    cg0 = cg_order[ci][0]
                    nc.scalar.activation(slab[:, cg0:cg0 + 2, :], ps[:], Exp,
                                         scale=1.0 / 16.0)

                for half in range(RC // 128):
                    rb = rc * (RC // 128) + half
                    nps = num_ps.tile([128, VA], f32, tag="num")
                    for i, (cg, _, _, _) in enumerate(cg_order):
                        nc.tensor.matmul(
                            nps[:], slab[:, cg, half * 128:(half + 1) * 128],
                            vaug_sb[:, cg, :],
                            start=(i == 0), stop=(i == NCH - 1))
                    rec = sc3.tile([128, 1], f32, tag="rec")
                    nc.vector.reciprocal(rec[:], nps[:, D:D + 1])
                    attn_bf = sc3.tile([128, D], bf16, tag="attn")
                    nc.vector.tensor_scalar_mul(attn_bf[:], nps[:, 0:D], rec[:])
                    attnT = sc3.tile([128, KC, 128], bf16, tag="attnT")
                    for j in range(KC):
                        tp = tp2_ps.tile([128, 128], bf16, tag="tp2")
                        nc.tensor.transpose(
                            tp[:], attn_bf[:, j * 128:(j + 1) * 128], ident_bf[:])
                        nc.vector.tensor_copy(attnT[:, j, :], tp[:])

                    hg = sm_ps.tile([128, F], f32, tag="smz")
                    for j in range(KC):
                        nc.tensor.matmul(hg[:, 0:D], attnT[:, j, :], wo_sb[:, j, :],
                                         start=(j == 0), stop=False)
                    nc.tensor.matmul(hg[:, 0:D], ones_k1[:], bor_sb[:],
                                     start=False, stop=True)
                    x2 = sc3.tile([128, D], f32, tag="x2")
                    nc.vector.tensor_add(x2[:], hg[:, 0:D], h1_sb[:, rb, :])
                    h2 = sc3.tile([128, D], f32, tag="h2")
                    _ln_block(nc, sc3, x2[:], h2[:],
                              gb_sb["g2"][:], gb_sb["be2"][:], eps_t[:])
                    h2bf = sc3.tile([128, D], bf16, tag="h2bf")
                    nc.vector.tensor_copy(h2bf[:], h2[:])
                    h2T = sc3.tile([128, KC, 128], bf16, tag="h2T")
                    for j in range(KC):
                        tp = tp2_ps.tile([128, 128], bf16, tag="tp2")
                        nc.tensor.transpose(
                            tp[:], h2bf[:, j * 128:(j + 1) * 128], ident_bf[:])
                        nc.vector.tensor_copy(h2T[:, j, :], tp[:])

                    ups = sm_ps.tile([128, F], f32, tag="smz")
                    for j in range(KC):
                        nc.tensor.matmul(ups[:], h2T[:, j, :], w1_sb[:, j, :],
                                         start=(j == 0), stop=False)
                    nc.tensor.matmul(ups[:], ones_k1[:], b1r_sb[:],
                                     start=False, stop=True)
                    u_sb = sc3.tile([128, F], bf16, tag="usb")
                    # ReLU on DVE (keeps ACT tables on exp/sqrt only)
                    nc.vector.tensor_scalar_max(u_sb[:], ups[:], 0.0)
                    uT = sc3.tile([128, FC, 128], bf16, tag="uT")
                    for jf in range(FC):
                        tp = tp2_ps.tile([128, 128], bf16, tag="tp2")
                        nc.tensor.transpose(
                            tp[:], u_sb[:, jf * 128:(jf + 1) * 128], ident_bf[:])
                        nc.vector.tensor_copy(uT[:, jf, :], tp[:])

                    o2 = sm_ps.tile([128, F], f32, tag="smz")
                    for jf in range(FC):
                        nc.tensor.matmul(o2[:, 0:D], uT[:, jf, :], w2_sb[:, jf, :],
                                         start=(jf == 0), stop=False)
                    nc.tensor.matmul(o2[:, 0:D], ones_k1[:], b2r_sb[:],
                                     start=False, stop=True)
                    x3 = sc3.tile([128, D], f32, tag="x3")
                    nc.vector.tensor_add(x3[:], o2[:, 0:D], h2[:])
                    o_sb = sc3.tile([128, D], f32, tag="osb")
                    _ln_block(nc, sc3, x3[:], o_sb[:],
                              gb_sb["g3"][:], gb_sb["be3"][:], eps_t[:])
                    nc.sync.dma_start(out_h[rb * 128:(rb + 1) * 128, :], o_sb[:])


_NC_CACHE = None


def _get_nc():
    global _NC_CACHE
    if _NC_CACHE is None:
        _NC_CACHE = _build()
    return _NC_CACHE


def _host_prep(inputs):
    """Build per-core in_maps from full inputs."""
    h = np.ascontiguousarray(np.asarray(inputs["h"], dtype=np.float32))
    ei = np.asarray(inputs["edge_index"]).astype(np.int64)
    src, dst = ei[0], ei[1]

    deg = np.bincount(dst, minlength=N).astype(np.float32) + 1.0
    dinv = 1.0 / np.sqrt(deg)
    coef = (dinv[src] * dinv[dst]).astype(np.float32)
    A = np.zeros((N, N), np.float32)
    np.add.at(A, (dst, src), coef)
    idx = np.arange(N)
    A[idx, idx] += dinv * dinv

    f32c = lambda k: np.ascontiguousarray(np.asarray(inputs[k], dtype=np.float32))
    bfc = lambda x: np.ascontiguousarray(x.astype(BF))

    ht = bfc(h.T)
    w = {k: f32c(k) for k in ("W_gcn", "Wq", "Wk", "Wv", "Wo", "RF",
                              "W1", "W2", "b_gcn", "bq", "bk", "bv", "bo",
                              "b1", "b2", "g1", "be1", "g2", "be2", "g3", "be3")}

    common = {
        "ht": ht,
        "wgcn": bfc(w["W_gcn"]), "wq": bfc(w["Wq"]), "wk": bfc(w["Wk"]),
        "wv": bfc(w["Wv"]), "wo": bfc(w["Wo"]), "rft": bfc(w["RF"].T),
        "w1": bfc(w["W1"]), "w2": bfc(w["W2"]),
        "bqc": np.ascontiguousarray(w["bq"].reshape(D, 1)),
        "bkc": np.ascontiguousarray(w["bk"].reshape(D, 1)),
        "bvr": bfc(w["bv"].reshape(1, D)),
        "bor": bfc(w["bo"].reshape(1, D)),
        "b1r": bfc(w["b1"].reshape(1, F)),
        "b2r": bfc(w["b2"].reshape(1, D)),
        "g1": np.ascontiguousarray(w["g1"].reshape(1, D)),
        "be1": np.ascontiguousarray(w["be1"].reshape(1, D)),
        "g2": np.ascontiguousarray(w["g2"].reshape(1, D)),
        "be2": np.ascontiguousarray(w["be2"].reshape(1, D)),
        "g3": np.ascontiguousarray(w["g3"].reshape(1, D)),
        "be3": np.ascontiguousarray(w["be3"].reshape(1, D)),
    }

    in_maps = []
    for c in range(NCORES):
        r0 = c * R
        # at[rb, p, k, f] = A[r0 + rb*128 + f, k*128 + p]
        a_loc = A[r0:r0 + R].reshape(RB, 128, NCH, 128)
        at = np.ascontiguousarray(a_loc.transpose(0, 3, 2, 1).astype(BF))
        m = dict(common)
        m["at"] = at
        m["hres"] = np.ascontiguousarray(h[r0:r0 + R] + w["b_gcn"])
        in_maps.append(m)
    return in_maps


def kernel(**inputs):
    nc = _get_nc()
    in_maps = _host_prep(inputs)
    res = run_bass_kernel_spmd(nc, in_maps, core_ids=list(range(NCORES)))
    out = np.concatenate([np.asarray(r["out"]) for r in res.results], axis=0)
    return out.astype(np.float32)
